# revision 5
# baseline (speedup 1.0000x reference)
"""BertSelfAttention on 8 Trainium2 NeuronCores (Bass/Tile, SPMD, no collectives).

Problem: hidden_states [2, 2048, 1024], 16 heads x 64 dims, causal_bias added
along the key axis before softmax.

Sharding: core c handles batch b = c//4 and head-group g = c%4 (4 heads, i.e.
256 of the 1024 projection dims).  Pure SPMD - every core runs the same
program on its own slice; the host does the (free) slicing / transposes and
the final gather.

Per-core device algorithm (all matmul operands bf16, accumulation f32):
  QT[m, s] = Wq_g @ hsT + bq   (m = 256 local head dims, s = 2048 positions)
  KT[m, s] = Wk_g @ hsT + bk
  V [s, m] = (hs @ Wv_g.T) * expb[s]   (expb = exp(causal_bias), no bv)
  per head-pair p (heads 2p, 2p+1 live on SBUF partitions 0:64 / 64:128 of
  QT[p]/KT[p]; the two scores matmuls are 64x128 row-tiles T0/T8 and run
  CONCURRENTLY on the PE array):
    per query block sq (512) and key chunk kk (128):
      sT[k, j, sq] = KT_h.T @ QT_h        (j = head-in-pair, 2 psum banks)
      P  [k, j, sq] = exp(sT * 0.125)     (ONE ACT instr per kk, bf16 out;
                                           bias folded via expb; |s/8| < ~3)
      cA/cB[65, sq] += [V'_h | expb].T @ P_h   (row 64 = softmax denominator)
  DMA ctxu to DRAM.
Host: ctx = (ctxu[:64] / ctxu[64]).T + bv  and scatter into [B, S, H].

Pipeline notes (the baseline at ~236us measured was ACT-duty-limited):
- The scalar engine (exp) has a hard floor of 16.8M elems / 128 lanes /
  1.2 GHz = 109us + per-instr access overhead ~= 133us per core; the PE floor
  is ~137us (proj 41 + row-tiled scores ~27 + PV 54.6 + mode switches).
- Scores psum is double-buffered ([128,2,512] tiles, 4 banks) so ACT runs
  back-to-back; the baseline's single-buffered scores cost ~584ns/iter of
  ACT idle (~37us).
- PV consumes the PREVIOUS group's P tiles, so each group's PE stream is
  [scores (64x128-tile mode) x2 | PV + projection chains (128 mode)] - one
  tiling-mode-switch pair per 2 key chunks.
- Projection chains are emitted through generators in ~4-matmul slices with
  due-slots, so the PE fills its slack under the ACT-bound attention without
  ever starving the scalar engine for more than one group.

The exp(bias) folding works because softmax(s + cb)_k = exp(s_k)*exp(cb_k) /
sum_k' exp(s_k')*exp(cb_k'), so scaling V rows and the denominator by
exp(cb_k) is exactly the bias add.
"""

import numpy as np

import concourse.tile as tile
from concourse import bacc, bass_utils, mybir

F32 = mybir.dt.float32
BF16 = mybir.dt.bfloat16
AF = mybir.ActivationFunctionType

B, S, H = 2, 2048, 1024
NH, HD = 16, 64
M = 256          # per-core projection dims (4 heads)
KC = H // 128    # 8 contraction chunks for the projections
ST = S // 128    # 16 key-position chunks
N_CORES = 8

_NC_CACHE = {}


def _attention_kernel(tc, reps=1, mode="full"):
    nc = tc.nc
    hsT = nc.dram_tensor("hsT", [H, S], BF16, kind="ExternalInput").ap()
    W3T = nc.dram_tensor("W3T", [H, 3 * M], BF16, kind="ExternalInput").ap()
    smalls = nc.dram_tensor("smalls", [128, 4 + ST], F32, kind="ExternalInput").ap()
    ctxu = nc.dram_tensor("ctxu", [4, HD + 1, S], F32, kind="ExternalOutput").ap()

    for _rep in range(reps):
      with (
        tc.tile_pool(name="const", bufs=1) as const,
        tc.tile_pool(name="big", bufs=1) as big,
      ):
        sm_sb = const.tile([128, 4 + ST], F32, tag="smalls", name="smalls")
        bq_sb = sm_sb[:, 0:2]
        bk_sb = sm_sb[:, 2:4]
        expb_sb = sm_sb[:, 4:4 + ST]
        ones_sb = const.tile([128, 4], F32, tag="ones", name="ones")
        nc.vector.memset(ones_sb[:], 1.0)

        hsT_big = big.tile([128, KC, S], BF16, tag="hsT", name="hsT_sb")
        hsT_r = hsT.rearrange("(c p) s -> p c s", p=128)
        w3_big = big.tile([128, KC, 3 * M], BF16, tag="w3", name="w3_sb")
        w3_r = W3T.rearrange("(c p) m -> p c m", p=128)
        # DMA staging across the two HWDGE rings (SP + ACT), ordered so the
        # first projection chains can start as early as possible: Wk first on
        # ring A, then hsT s-quarters alternating between rings (a chain for
        # s-block q needs ALL hidden chunks of that block, so quarters are
        # loaded s-major).  smalls is tiny and first on ring B (biases are
        # needed by the first chain's DVE add).
        nc.scalar.dma_start(out=sm_sb[:], in_=smalls[:])
        nc.sync.dma_start(out=w3_big[:, :, M:2 * M], in_=w3_r[:, :, M:2 * M])
        nc.scalar.dma_start(out=w3_big[:, :, 0:M], in_=w3_r[:, :, 0:M])
        nc.scalar.dma_start(out=w3_big[:, :, 2 * M:3 * M], in_=w3_r[:, :, 2 * M:3 * M])
        # the first s-quarter is split by hidden-chunk halves so the first K
        # chain's early matmuls can start before the whole quarter lands
        nc.sync.dma_start(out=hsT_big[:, 0:4, 0:512], in_=hsT_r[:, 0:4, 0:512])
        nc.sync.dma_start(out=hsT_big[:, 4:8, 0:512], in_=hsT_r[:, 4:8, 0:512])
        nc.scalar.dma_start(out=hsT_big[:, :, 512:1024], in_=hsT_r[:, :, 512:1024])
        nc.sync.dma_start(out=hsT_big[:, :, 1024:1536], in_=hsT_r[:, :, 1024:1536])
        nc.scalar.dma_start(out=hsT_big[:, :, 1536:2048], in_=hsT_r[:, :, 1536:2048])
        # ACT warmup: a dummy Exp right after the ring-B DMA dispatches pulls
        # the ~2.7us activation-table load into the DMA wait window
        wact = const.tile([128, 1], F32, tag="wact", name="wact")
        nc.scalar.activation(wact[:], ones_sb[:, 0:1], AF.Exp, scale=0.125)

        # Persistent projection outputs (bf16).
        QT = [big.tile([128, S], BF16, tag=f"QT{t}", name=f"QT{t}") for t in range(2)]
        KT = [big.tile([128, S], BF16, tag=f"KT{t}", name=f"KT{t}") for t in range(2)]
        # V' with exp(bias) column interleaved: per key chunk, 4 head blocks
        # of [64 scaled V dims | expb] = 260 columns.
        Vp = [big.tile([128, 4, HD + 1], BF16, tag=f"Vp{s}", name=f"Vp{s}") for s in range(ST)]

        with (
            tc.tile_pool(name="pp", bufs=2, space="PSUM") as pp,        # 2 banks
            tc.tile_pool(name="sc", bufs=2, space="PSUM") as sc_pool,   # 4 banks
            tc.tile_pool(name="cx", bufs=1, space="PSUM") as cx_pool,   # 2 banks
            tc.tile_pool(name="pt", bufs=4) as pt_pool,
            tc.tile_pool(name="cs", bufs=2) as cs_pool,
        ):

            def qk_chain_gen(col0, out_t, bias_col, mt, sc):
                ps = pp.tile([128, 512], F32, tag="pp", name="qk")
                for k in range(KC):
                    nc.tensor.matmul(
                        ps[:],
                        w3_big[:, k, col0 + mt * 128:col0 + (mt + 1) * 128],
                        hsT_big[:, k, sc * 512:(sc + 1) * 512],
                        start=(k == 0),
                        stop=(k == KC - 1),
                    )
                    if k == 3:
                        yield
                nc.vector.tensor_scalar_add(
                    out_t[mt][:, sc * 512:(sc + 1) * 512],
                    ps[:],
                    sm_sb[:, bias_col + mt:bias_col + mt + 1],
                )

            def v_chain_gen(st):
                ps = pp.tile([128, M], F32, tag="pp", name="v")
                for k in range(KC):
                    nc.tensor.matmul(
                        ps[:],
                        hsT_big[:, k, st * 128:(st + 1) * 128],
                        w3_big[:, k, 2 * M:3 * M],
                        start=(k == 0),
                        stop=(k == KC - 1),
                    )
                    if k == 3:
                        yield
                nc.vector.tensor_scalar_mul(
                    Vp[st][:, :, 0:HD],
                    ps[:].rearrange("p (h d) -> p h d", h=4),
                    expb_sb[:, st:st + 1],
                )
                nc.vector.tensor_scalar_mul(
                    Vp[st][:, :, HD:HD + 1],
                    ones_sb[:].rearrange("p (h d) -> p h d", h=4),
                    expb_sb[:, st:st + 1],
                )

            def K_gen(p_, sc):
                return qk_chain_gen(M, KT, 2, p_, sc)

            def Q_gen(p_, sc):
                return qk_chain_gen(0, QT, 0, p_, sc)

            def run(gen):
                for _ in gen:
                    pass

            if mode == "dmaonly":
                dummy = const.tile([128, 1], BF16, tag="dummy", name="dummy")
                nc.vector.tensor_copy(dummy[:], hsT_big[:, 0, 0:1])
                nc.vector.tensor_copy(dummy[:], w3_big[:, 0, 0:1])
                continue

            # Minimal prefix so (pair 0, sq-block 0, key chunks 0..3) can
            # start as soon as the first hsT s-quarter lands.  V chains are
            # NOT in the prefix: PV lags scores by a group, so V0/V1 are only
            # needed one group in - keeping them out starts the first exp
            # (the critical engine) ~3.4us earlier.
            run(K_gen(0, 0))
            run(Q_gen(0, 0))

            # Background queue of remaining projection work, as (due-slot,
            # generator).  Slots number the 64 attention groups globally;
            # an item with due d is fully emitted during group d's 128-mode
            # phase at the latest, i.e. before any consumer in group d+1.
            bg = [
                (0, v_chain_gen(0)), (0, v_chain_gen(1)),
                (1, v_chain_gen(2)), (1, v_chain_gen(3)),
                (1, K_gen(0, 1)),
                (2, v_chain_gen(4)), (2, v_chain_gen(5)),
                (3, v_chain_gen(6)), (3, v_chain_gen(7)), (3, K_gen(0, 2)),
                (4, v_chain_gen(8)), (4, v_chain_gen(9)),
                (5, v_chain_gen(10)), (5, v_chain_gen(11)), (5, K_gen(0, 3)),
                (6, v_chain_gen(12)), (6, v_chain_gen(13)),
                (7, v_chain_gen(14)), (7, v_chain_gen(15)), (7, Q_gen(0, 1)),
                (15, Q_gen(0, 2)),
                (23, Q_gen(0, 3)),
                (31, K_gen(1, 0)), (31, Q_gen(1, 0)),
                (33, K_gen(1, 1)),
                (35, K_gen(1, 2)),
                (37, K_gen(1, 3)),
                (39, Q_gen(1, 1)),
                (47, Q_gen(1, 2)),
                (55, Q_gen(1, 3)),
            ]

            def drain_due(slot, extra=0):
                # emit everything due, then up to `extra` more ~850ns slices
                while bg and bg[0][0] <= slot:
                    try:
                        next(bg[0][1])
                    except StopIteration:
                        bg.pop(0)
                for _ in range(extra):
                    if not bg:
                        return
                    try:
                        next(bg[0][1])
                    except StopIteration:
                        bg.pop(0)

            if mode == "projonly":
                while bg:
                    drain_due(10 ** 9)
                continue

            def emit_pv(ctx_, kk, pT):
                cA_, cB_, p_, _sqc = ctx_
                fl = dict(start=(kk == 0), stop=(kk == ST - 1))
                nc.tensor.matmul(cA_[:], Vp[kk][:, 2 * p_, :], pT[:, 0, :], **fl)
                nc.tensor.matmul(cB_[:], Vp[kk][:, 2 * p_ + 1, :], pT[:, 1, :], **fl)

            def emit_flush(ctx_, pend_):
                # trailing PVs of a finished sq-block + psum evacuation + DMA
                for kk, pT in pend_:
                    emit_pv(ctx_, kk, pT)
                cA_, cB_, p_, sqc_ = ctx_
                sq_ = slice(sqc_ * 512, (sqc_ + 1) * 512)
                o2 = cs_pool.tile([HD + 1, 2, 512], F32, tag="o2", name="o2")
                nc.vector.tensor_copy(o2[:, 0, :], cA_[:])
                nc.vector.tensor_copy(o2[:, 1, :], cB_[:])
                # one DMA for both heads: DRAM side takes the head axis as a
                # stride
                nc.sync.dma_start(
                    out=ctxu[2 * p_:2 * p_ + 2, :, sq_].rearrange("h p c -> p h c"),
                    in_=o2[:],
                )

            # One flat software-pipelined stream over all 64 groups: a
            # finished block's trailing PVs/evacuation are emitted AFTER the
            # next block's first scores+exp, so the scalar engine never
            # bubbles at block boundaries.
            pend = []
            cur_ctx = None
            for slot in range(64):
                p, rem = divmod(slot, 32)
                sqc, g = divmod(rem, 8)
                sq = slice(sqc * 512, (sqc + 1) * 512)
                flush = None
                if g == 0 and mode == "full":
                    flush = (cur_ctx, pend)
                    pend = []
                    cA = cx_pool.tile([HD + 1, 512], F32, tag="cA", name="cA")
                    cB = cx_pool.tile([HD + 1, 512], F32, tag="cB", name="cB")
                    cur_ctx = (cA, cB, p, sqc)
                cur = []
                for i in range(2):
                    kk = 2 * g + i
                    ks = slice(kk * 128, (kk + 1) * 128)
                    sT = sc_pool.tile([128, 2, 512], F32, tag="s", name="s")
                    # 64x128 row-tiles T0/T8: concurrent on the PE
                    nc.tensor.matmul(sT[:, 0, :], KT[p][0:64, ks], QT[p][0:64, sq])
                    nc.tensor.matmul(sT[:, 1, :], KT[p][64:128, ks], QT[p][64:128, sq])
                    if mode == "scoresonly":
                        dmy = pt_pool.tile([128, 1], F32, tag="dmy", name="dmy")
                        nc.vector.tensor_copy(dmy[:], sT[:, 0, 0:1])
                        continue
                    pT = pt_pool.tile([128, 2, 512], BF16, tag="p", name="p")
                    nc.scalar.activation(pT[:], sT[:], AF.Exp, scale=0.125)
                    cur.append((kk, pT))
                if mode == "full":
                    if flush is not None:
                        if flush[0] is not None:
                            emit_flush(flush[0], flush[1])
                    else:
                        for kk, pT in pend:
                            emit_pv(cur_ctx, kk, pT)
                pend = cur
                drain_due(slot, extra=1)
            if mode == "full":
                emit_flush(cur_ctx, pend)
            while bg:
                drain_due(10 ** 9)


def build_nc(reps=1, mode="full"):
    key = (reps, mode)
    if key in _NC_CACHE:
        return _NC_CACHE[key]
    nc = bacc.Bacc("TRN2", target_bir_lowering=False, debug=False)
    with tile.TileContext(nc) as tc:
        _attention_kernel(tc, reps=reps, mode=mode)
    nc.compile()
    _NC_CACHE[key] = nc
    return nc


def make_in_maps(hidden_states, causal_bias, Wq, bq, Wk, bk, Wv, bv):
    bf16 = mybir.dt.np(BF16)
    hs = np.asarray(hidden_states, dtype=np.float32)
    cb = np.asarray(causal_bias, dtype=np.float32)
    expb = np.exp(cb).reshape(ST, 128).T.copy()  # [128, ST]
    hsT = [np.ascontiguousarray(hs[b].T).astype(bf16) for b in range(B)]
    in_maps = []
    for c in range(N_CORES):
        b, g = divmod(c, 4)
        sl = slice(g * M, (g + 1) * M)
        w3 = np.concatenate([
            np.asarray(Wq, np.float32)[sl].T,
            np.asarray(Wk, np.float32)[sl].T,
            np.asarray(Wv, np.float32)[sl].T,
        ], axis=1).astype(bf16)
        sm = np.concatenate([
            np.asarray(bq, np.float32)[sl].reshape(2, 128).T,
            np.asarray(bk, np.float32)[sl].reshape(2, 128).T,
            expb,
        ], axis=1)
        in_maps.append({
            "hsT": hsT[b],
            "W3T": np.ascontiguousarray(w3),
            "smalls": np.ascontiguousarray(sm),
        })
    return in_maps


def gather_output(results, bv):
    bv = np.asarray(bv, np.float32)
    out = np.empty((B, S, H), np.float32)
    for c in range(N_CORES):
        b, g = divmod(c, 4)
        sl = slice(g * M, (g + 1) * M)
        ctxu = results[c]["ctxu"]  # [4, 65, S]
        ctx = (ctxu[:, :HD, :] / ctxu[:, HD:HD + 1, :]).transpose(2, 0, 1)
        out[b, :, sl] = ctx.reshape(S, M) + bv[sl][None, :]
    return out


def kernel(hidden_states, causal_bias, Wq, bq, Wk, bk, Wv, bv):
    nc = build_nc()
    in_maps = make_in_maps(hidden_states, causal_bias, Wq, bq, Wk, bk, Wv, bv)
    res = bass_utils.run_bass_kernel_spmd(nc, in_maps, core_ids=list(range(N_CORES)))
    return gather_output(res.results, bv)


# revision 7
# speedup vs baseline: 1.0124x; 1.0124x over previous
"""BertSelfAttention on 8 Trainium2 NeuronCores (Bass/Tile, SPMD, no collectives).

Problem: hidden_states [2, 2048, 1024], 16 heads x 64 dims, causal_bias added
along the key axis before softmax.

Sharding: core c handles batch b = c//4 and head-group g = c%4 (4 heads, i.e.
256 of the 1024 projection dims).  Pure SPMD - every core runs the same
program on its own slice; the host does the (free) slicing / transposes and
the final gather.

Per-core device algorithm (all matmul operands bf16, accumulation f32):
  QT[m, s] = Wq_g @ hsT + bq   (m = 256 local head dims, s = 2048 positions)
  KT[m, s] = Wk_g @ hsT + bk
  V [s, m] = (hs @ Wv_g.T) * expb[s]   (expb = exp(causal_bias), no bv)
  per head-pair p (heads 2p, 2p+1 live on SBUF partitions 0:64 / 64:128 of
  QT[p]/KT[p]; the two scores matmuls are 64x128 row-tiles T0/T8 and run
  CONCURRENTLY on the PE array):
    per query block sq (512) and key chunk kk (128):
      sT[k, j, sq] = KT_h.T @ QT_h        (j = head-in-pair, 2 psum banks)
      P  [k, j, sq] = exp(sT * 0.125)     (ONE ACT instr per kk, bf16 out;
                                           bias folded via expb; |s/8| < ~3)
      cA/cB[65, sq] += [V'_h | expb].T @ P_h   (row 64 = softmax denominator)
  DMA ctxu to DRAM.
Host: ctx = (ctxu[:64] / ctxu[64]).T + bv  and scatter into [B, S, H].

Pipeline notes (the baseline at ~236us measured was ACT-duty-limited):
- The scalar engine (exp) has a hard floor of 16.8M elems / 128 lanes /
  1.2 GHz = 109us + per-instr access overhead ~= 133us per core; the PE floor
  is ~137us (proj 41 + row-tiled scores ~27 + PV 54.6 + mode switches).
- Scores psum is double-buffered ([128,2,512] tiles, 4 banks) so ACT runs
  back-to-back; the baseline's single-buffered scores cost ~584ns/iter of
  ACT idle (~37us).
- PV consumes the PREVIOUS group's P tiles, so each group's PE stream is
  [scores (64x128-tile mode) x2 | PV + projection chains (128 mode)] - one
  tiling-mode-switch pair per 2 key chunks.
- Projection chains are emitted through generators in ~4-matmul slices with
  due-slots, so the PE fills its slack under the ACT-bound attention without
  ever starving the scalar engine for more than one group.

The exp(bias) folding works because softmax(s + cb)_k = exp(s_k)*exp(cb_k) /
sum_k' exp(s_k')*exp(cb_k'), so scaling V rows and the denominator by
exp(cb_k) is exactly the bias add.
"""

import numpy as np

import concourse.tile as tile
from concourse import bacc, bass_utils, mybir

F32 = mybir.dt.float32
BF16 = mybir.dt.bfloat16
AF = mybir.ActivationFunctionType

B, S, H = 2, 2048, 1024
NH, HD = 16, 64
M = 256          # per-core projection dims (4 heads)
KC = H // 128    # 8 contraction chunks for the projections
ST = S // 128    # 16 key-position chunks
N_CORES = 8

_NC_CACHE = {}


def _attention_kernel(tc, reps=1, mode="full"):
    nc = tc.nc
    hsT = nc.dram_tensor("hsT", [H, S], BF16, kind="ExternalInput").ap()
    W3T = nc.dram_tensor("W3T", [H, 3 * M], BF16, kind="ExternalInput").ap()
    smalls = nc.dram_tensor("smalls", [128, 4 + ST], F32, kind="ExternalInput").ap()
    ctxu = nc.dram_tensor("ctxu", [4, HD + 1, S], F32, kind="ExternalOutput").ap()

    for _rep in range(reps):
      with (
        tc.tile_pool(name="const", bufs=1) as const,
        tc.tile_pool(name="big", bufs=1) as big,
      ):
        sm_sb = const.tile([128, 4 + ST], F32, tag="smalls", name="smalls")
        bq_sb = sm_sb[:, 0:2]
        bk_sb = sm_sb[:, 2:4]
        expb_sb = sm_sb[:, 4:4 + ST]
        ones_sb = const.tile([128, 4], F32, tag="ones", name="ones")
        nc.vector.memset(ones_sb[:], 1.0)

        hsT_big = big.tile([128, KC, S], BF16, tag="hsT", name="hsT_sb")
        hsT_r = hsT.rearrange("(c p) s -> p c s", p=128)
        w3_big = big.tile([128, KC, 3 * M], BF16, tag="w3", name="w3_sb")
        w3_r = W3T.rearrange("(c p) m -> p c m", p=128)
        # DMA staging across the two HWDGE rings (SP + ACT), ordered so the
        # first projection chains can start as early as possible: Wk first on
        # ring A, then hsT s-quarters alternating between rings (a chain for
        # s-block q needs ALL hidden chunks of that block, so quarters are
        # loaded s-major).  smalls is tiny and first on ring B (biases are
        # needed by the first chain's DVE add).
        nc.scalar.dma_start(out=sm_sb[:], in_=smalls[:])
        nc.sync.dma_start(out=w3_big[:, :, M:2 * M], in_=w3_r[:, :, M:2 * M])
        nc.scalar.dma_start(out=w3_big[:, :, 0:M], in_=w3_r[:, :, 0:M])
        nc.scalar.dma_start(out=w3_big[:, :, 2 * M:3 * M], in_=w3_r[:, :, 2 * M:3 * M])
        # the first s-quarter is split by hidden-chunk halves so the first K
        # chain's early matmuls can start before the whole quarter lands
        nc.sync.dma_start(out=hsT_big[:, 0:4, 0:512], in_=hsT_r[:, 0:4, 0:512])
        nc.sync.dma_start(out=hsT_big[:, 4:8, 0:512], in_=hsT_r[:, 4:8, 0:512])
        nc.scalar.dma_start(out=hsT_big[:, :, 512:1024], in_=hsT_r[:, :, 512:1024])
        nc.sync.dma_start(out=hsT_big[:, :, 1024:1536], in_=hsT_r[:, :, 1024:1536])
        nc.scalar.dma_start(out=hsT_big[:, :, 1536:2048], in_=hsT_r[:, :, 1536:2048])
        # ACT warmup: a dummy Exp right after the ring-B DMA dispatches pulls
        # the ~2.7us activation-table load into the DMA wait window
        wact = const.tile([128, 1], F32, tag="wact", name="wact")
        nc.scalar.activation(wact[:], ones_sb[:, 0:1], AF.Exp, scale=0.125)

        # Persistent projection outputs (bf16).
        QT = [big.tile([128, S], BF16, tag=f"QT{t}", name=f"QT{t}") for t in range(2)]
        KT = [big.tile([128, S], BF16, tag=f"KT{t}", name=f"KT{t}") for t in range(2)]
        # V' with exp(bias) column interleaved: per key chunk, 4 head blocks
        # of [64 scaled V dims | expb] = 260 columns.
        Vp = [big.tile([128, 4, HD + 1], BF16, tag=f"Vp{s}", name=f"Vp{s}") for s in range(ST)]

        with (
            tc.tile_pool(name="pp", bufs=2, space="PSUM") as pp,        # 2 banks
            tc.tile_pool(name="sc", bufs=2, space="PSUM") as sc_pool,   # 4 banks
            tc.tile_pool(name="cx", bufs=1, space="PSUM") as cx_pool,   # 2 banks
            tc.tile_pool(name="pt", bufs=4) as pt_pool,
            tc.tile_pool(name="cs", bufs=2) as cs_pool,
        ):

            def qk_chain_gen(col0, out_t, bias_col, mt, sc):
                ps = pp.tile([128, 512], F32, tag="pp", name="qk")
                for k in range(KC):
                    nc.tensor.matmul(
                        ps[:],
                        w3_big[:, k, col0 + mt * 128:col0 + (mt + 1) * 128],
                        hsT_big[:, k, sc * 512:(sc + 1) * 512],
                        start=(k == 0),
                        stop=(k == KC - 1),
                    )
                    if k == 3:
                        yield
                nc.vector.tensor_scalar_add(
                    out_t[mt][:, sc * 512:(sc + 1) * 512],
                    ps[:],
                    sm_sb[:, bias_col + mt:bias_col + mt + 1],
                )

            def v_chain_gen(st):
                ps = pp.tile([128, M], F32, tag="pp", name="v")
                for k in range(KC):
                    nc.tensor.matmul(
                        ps[:],
                        hsT_big[:, k, st * 128:(st + 1) * 128],
                        w3_big[:, k, 2 * M:3 * M],
                        start=(k == 0),
                        stop=(k == KC - 1),
                    )
                    if k == 3:
                        yield
                nc.vector.tensor_scalar_mul(
                    Vp[st][:, :, 0:HD],
                    ps[:].rearrange("p (h d) -> p h d", h=4),
                    expb_sb[:, st:st + 1],
                )
                nc.vector.tensor_scalar_mul(
                    Vp[st][:, :, HD:HD + 1],
                    ones_sb[:].rearrange("p (h d) -> p h d", h=4),
                    expb_sb[:, st:st + 1],
                )

            def K_gen(p_, sc):
                return qk_chain_gen(M, KT, 2, p_, sc)

            def Q_gen(p_, sc):
                return qk_chain_gen(0, QT, 0, p_, sc)

            def run(gen):
                for _ in gen:
                    pass

            if mode != "dmaonly":
                # PE HAM warmup: the PE clock-gate defaults to 1.2 GHz and
                # un-throttles to 2.4 GHz only after ~3.4us of sustained
                # activity.  A stream of tiny matmuls on a zeroed tile during
                # the initial DMA wait burns the cold window so the first
                # real projection chains run warm.
                wmt = const.tile([128, 80], BF16, tag="wmt", name="wmt")
                nc.vector.memset(wmt[:], 0.0)
                wps = pp.tile([65, 48], F32, tag="pp", name="warm")
                for _ in range(64):
                    nc.tensor.matmul(wps[:], wmt[:, 0:65], wmt[:, 0:48])

            if mode == "dmaonly":
                dummy = const.tile([128, 1], BF16, tag="dummy", name="dummy")
                nc.vector.tensor_copy(dummy[:], hsT_big[:, 0, 0:1])
                nc.vector.tensor_copy(dummy[:], w3_big[:, 0, 0:1])
                continue

            # Minimal prefix so (pair 0, sq-block 0, key chunks 0..3) can
            # start as soon as the first hsT s-quarter lands.  V chains are
            # NOT in the prefix: PV lags scores by a group, so V0/V1 are only
            # needed one group in - keeping them out starts the first exp
            # (the critical engine) ~3.4us earlier.
            run(K_gen(0, 0))
            run(Q_gen(0, 0))

            # Background queue of remaining projection work, as (due-slot,
            # generator).  Slots number the 64 attention groups globally;
            # an item with due d is fully emitted during group d's 128-mode
            # phase at the latest, i.e. before any consumer in group d+1.
            bg = [
                (0, v_chain_gen(0)), (0, v_chain_gen(1)),
                (1, v_chain_gen(2)), (1, v_chain_gen(3)),
                (1, K_gen(0, 1)),
                (2, v_chain_gen(4)), (2, v_chain_gen(5)),
                (3, v_chain_gen(6)), (3, v_chain_gen(7)), (3, K_gen(0, 2)),
                (4, v_chain_gen(8)), (4, v_chain_gen(9)),
                (5, v_chain_gen(10)), (5, v_chain_gen(11)), (5, K_gen(0, 3)),
                (6, v_chain_gen(12)), (6, v_chain_gen(13)),
                (7, v_chain_gen(14)), (7, v_chain_gen(15)), (7, Q_gen(0, 1)),
                (15, Q_gen(0, 2)),
                (23, Q_gen(0, 3)),
                (31, K_gen(1, 0)), (31, Q_gen(1, 0)),
                (33, K_gen(1, 1)),
                (35, K_gen(1, 2)),
                (37, K_gen(1, 3)),
                (39, Q_gen(1, 1)),
                (47, Q_gen(1, 2)),
                (55, Q_gen(1, 3)),
            ]

            def drain_due(slot, extra=0):
                # emit everything due, then up to `extra` more ~850ns slices
                while bg and bg[0][0] <= slot:
                    try:
                        next(bg[0][1])
                    except StopIteration:
                        bg.pop(0)
                for _ in range(extra):
                    if not bg:
                        return
                    try:
                        next(bg[0][1])
                    except StopIteration:
                        bg.pop(0)

            if mode == "projonly":
                while bg:
                    drain_due(10 ** 9)
                continue

            def emit_pv(ctx_, kk, pT):
                cA_, cB_, p_, _sqc = ctx_
                fl = dict(start=(kk == 0), stop=(kk == ST - 1))
                nc.tensor.matmul(cA_[:], Vp[kk][:, 2 * p_, :], pT[:, 0, :], **fl)
                nc.tensor.matmul(cB_[:], Vp[kk][:, 2 * p_ + 1, :], pT[:, 1, :], **fl)

            def emit_flush(ctx_, pend_):
                # trailing PVs of a finished sq-block + psum evacuation + DMA,
                # interleaved per head so cA's evacuation overlaps cB's PVs
                cA_, cB_, p_, sqc_ = ctx_
                sq_ = slice(sqc_ * 512, (sqc_ + 1) * 512)
                o2 = cs_pool.tile([HD + 1, 2, 512], F32, tag="o2", name="o2")
                for kk, pT in pend_:
                    fl = dict(start=(kk == 0), stop=(kk == ST - 1))
                    nc.tensor.matmul(cA_[:], Vp[kk][:, 2 * p_, :], pT[:, 0, :], **fl)
                nc.vector.tensor_copy(o2[:, 0, :], cA_[:])
                nc.sync.dma_start(out=ctxu[2 * p_, :, sq_], in_=o2[:, 0, :])
                for kk, pT in pend_:
                    fl = dict(start=(kk == 0), stop=(kk == ST - 1))
                    nc.tensor.matmul(cB_[:], Vp[kk][:, 2 * p_ + 1, :], pT[:, 1, :], **fl)
                nc.vector.tensor_copy(o2[:, 1, :], cB_[:])
                nc.sync.dma_start(out=ctxu[2 * p_ + 1, :, sq_], in_=o2[:, 1, :])

            # One flat software-pipelined stream over all 64 groups: a
            # finished block's trailing PVs/evacuation are emitted AFTER the
            # next block's first scores+exp, so the scalar engine never
            # bubbles at block boundaries.
            pend = []
            cur_ctx = None
            for slot in range(64):
                p, rem = divmod(slot, 32)
                sqc, g = divmod(rem, 8)
                sq = slice(sqc * 512, (sqc + 1) * 512)
                flush = None
                if g == 0 and mode == "full":
                    flush = (cur_ctx, pend)
                    pend = []
                    cA = cx_pool.tile([HD + 1, 512], F32, tag="cA", name="cA")
                    cB = cx_pool.tile([HD + 1, 512], F32, tag="cB", name="cB")
                    cur_ctx = (cA, cB, p, sqc)
                cur = []
                for i in range(2):
                    kk = 2 * g + i
                    ks = slice(kk * 128, (kk + 1) * 128)
                    sT = sc_pool.tile([128, 2, 512], F32, tag="s", name="s")
                    # 64x128 row-tiles T0/T8: concurrent on the PE
                    nc.tensor.matmul(sT[:, 0, :], KT[p][0:64, ks], QT[p][0:64, sq])
                    nc.tensor.matmul(sT[:, 1, :], KT[p][64:128, ks], QT[p][64:128, sq])
                    if mode == "scoresonly":
                        dmy = pt_pool.tile([128, 1], F32, tag="dmy", name="dmy")
                        nc.vector.tensor_copy(dmy[:], sT[:, 0, 0:1])
                        continue
                    pT = pt_pool.tile([128, 2, 512], BF16, tag="p", name="p")
                    nc.scalar.activation(pT[:], sT[:], AF.Exp, scale=0.125)
                    cur.append((kk, pT))
                if mode == "full":
                    if flush is not None:
                        if flush[0] is not None:
                            emit_flush(flush[0], flush[1])
                    else:
                        for kk, pT in pend:
                            emit_pv(cur_ctx, kk, pT)
                pend = cur
                drain_due(slot, extra=1)
            if mode == "full":
                emit_flush(cur_ctx, pend)
            while bg:
                drain_due(10 ** 9)


def build_nc(reps=1, mode="full"):
    key = (reps, mode)
    if key in _NC_CACHE:
        return _NC_CACHE[key]
    nc = bacc.Bacc("TRN2", target_bir_lowering=False, debug=False)
    with tile.TileContext(nc) as tc:
        _attention_kernel(tc, reps=reps, mode=mode)
    nc.compile()
    _NC_CACHE[key] = nc
    return nc


def make_in_maps(hidden_states, causal_bias, Wq, bq, Wk, bk, Wv, bv):
    bf16 = mybir.dt.np(BF16)
    hs = np.asarray(hidden_states, dtype=np.float32)
    cb = np.asarray(causal_bias, dtype=np.float32)
    expb = np.exp(cb).reshape(ST, 128).T.copy()  # [128, ST]
    hsT = [np.ascontiguousarray(hs[b].T).astype(bf16) for b in range(B)]
    in_maps = []
    for c in range(N_CORES):
        b, g = divmod(c, 4)
        sl = slice(g * M, (g + 1) * M)
        w3 = np.concatenate([
            np.asarray(Wq, np.float32)[sl].T,
            np.asarray(Wk, np.float32)[sl].T,
            np.asarray(Wv, np.float32)[sl].T,
        ], axis=1).astype(bf16)
        sm = np.concatenate([
            np.asarray(bq, np.float32)[sl].reshape(2, 128).T,
            np.asarray(bk, np.float32)[sl].reshape(2, 128).T,
            expb,
        ], axis=1)
        in_maps.append({
            "hsT": hsT[b],
            "W3T": np.ascontiguousarray(w3),
            "smalls": np.ascontiguousarray(sm),
        })
    return in_maps


def gather_output(results, bv):
    bv = np.asarray(bv, np.float32)
    out = np.empty((B, S, H), np.float32)
    for c in range(N_CORES):
        b, g = divmod(c, 4)
        sl = slice(g * M, (g + 1) * M)
        ctxu = results[c]["ctxu"]  # [4, 65, S]
        ctx = (ctxu[:, :HD, :] / ctxu[:, HD:HD + 1, :]).transpose(2, 0, 1)
        out[b, :, sl] = ctx.reshape(S, M) + bv[sl][None, :]
    return out


def kernel(hidden_states, causal_bias, Wq, bq, Wk, bk, Wv, bv):
    nc = build_nc()
    in_maps = make_in_maps(hidden_states, causal_bias, Wq, bq, Wk, bk, Wv, bv)
    res = bass_utils.run_bass_kernel_spmd(nc, in_maps, core_ids=list(range(N_CORES)))
    return gather_output(res.results, bv)
